# revision 30
# baseline (speedup 1.0000x reference)
"""MQA (GQA with 1 KV group) attention kernel for 8 Trainium2 NeuronCores.

Sharding: core c -> batch b = c//4, head-group hg = c%4 (4 of 16 query heads).
Each core computes Q/K/V projections from x[b]^T, causal attention for its 4
heads in transposed layout (S^T[kv, q] tiles), and a partial output
projection out_partial = A_h @ Wo[:, cols_h]^T.  Host sums the 4 partials per
batch and adds bo.

Structure (one pass over 4 s-chunks, fully interleaved emission so every
engine's queue overlaps the PE-bound projection work):
  - proj chunk sc (K,V -> one 2-bank PSUM tile; Q pairs likewise), then
    attention qc=sc, with the qc-1 normalization deferred into chunk sc's
    emission so no PSUM tag is blocked at chunk boundaries;
  - scores live in [128,1024] two-bank PSUM tiles (one exp per head-pair,
    same KT/V stationary reused across the pair);
  - the causal mask is accumulated onto scores by the PE (identity matmul)
    and score/exp/accumulate work is restricted to the valid causal
    q-suffix on diagonal tiles (GpSimd zeroes the dead es prefix);
  - exp outputs / softmax denominators are fp16 (2x DVE rate, full-rate
    rowsum matmul); 1/rowsum via reciprocal_approx_fast; the per-q scale is
    partition-broadcast on GpSimd.  All PSUM rowsums stay at partition 0:
    DVE lanes are partition-locked on HW, and gpsimd.partition_broadcast
    reads absolute partition 0 (offset-base APs silently break on HW even
    though CoreSim accepts them);
  - output projection runs as 32 half-tiles alternating the two score tags,
    bf16 partials summed on host;
  - weights are host-pre-transposed into per-partition-contiguous layouts
    (strided rearrange DMAs cost ~4x in descriptor overhead), and the wq
    chunks queue behind the first xT chunk so nothing stalls startup.
"""

import sys

sys.path.insert(0, "/opt/trn_rl_repo")

import ml_dtypes
import numpy as np

import concourse.bass as bass
import concourse.tile as tile
from concourse import bacc
from concourse import mybir
from concourse.bass import ts
from concourse.bass_utils import run_bass_kernel_spmd
from concourse.masks import make_identity

B, S, HID = 2, 2048, 2048
H, D = 16, 128
HPC = 4              # heads per core
DPH = HPC * D        # 512: head dims per core
NCORES = 8
SC = 512             # s-chunk (free dim for most matmuls)
NSC = S // SC        # 4
NT = S // 128        # 16 128-tiles along s / hid
NHT = HID // 128     # 16 hid tiles
SCALE = 1.0 / float(np.sqrt(D))
NEG = -1.0e9

F32 = mybir.dt.float32
F16 = mybir.dt.float16
BF16 = mybir.dt.bfloat16
NP_BF16 = ml_dtypes.bfloat16

_PROGRAM = None
LAST_RESULT = None


def _build_program():
    nc = bacc.Bacc()
    xT = nc.declare_dram_parameter("xT", [NSC, 128, NHT, SC], BF16, isOutput=False)
    wq = nc.declare_dram_parameter("wq", [HPC, 128, NHT, 128], BF16, isOutput=False)
    wk = nc.declare_dram_parameter("wk", [128, NHT, D], BF16, isOutput=False)
    wv = nc.declare_dram_parameter("wv", [128, NHT, D], BF16, isOutput=False)
    wo = nc.declare_dram_parameter("wo", [128, HPC, HID], BF16, isOutput=False)
    bq = nc.declare_dram_parameter("bq", [128, HPC], F32, isOutput=False)
    bkv = nc.declare_dram_parameter("bkv", [128, 2], F32, isOutput=False)
    padb = nc.declare_dram_parameter("padb", [128, NT], F32, isOutput=False)
    dmask = nc.declare_dram_parameter("dmask", [128, 4 * SC], BF16, isOutput=False)
    out = nc.declare_dram_parameter("out", [S, HID], BF16, isOutput=True)

    Exp = mybir.ActivationFunctionType.Exp
    Ident = mybir.ActivationFunctionType.Identity

    with tile.TileContext(nc) as tc:
        with (
            tc.tile_pool(name="consts", bufs=1) as consts,
            tc.tile_pool(name="persist", bufs=1) as persist,
        ):
            ident = consts.tile([128, 128], BF16)
            make_identity(nc, ident[:])
            ident16 = consts.tile([128, 128], F16)
            make_identity(nc, ident16[:])
            ones_col = consts.tile([128, 1], F16)
            nc.vector.memset(ones_col[:], 1.0)
            bq_sb = consts.tile([128, HPC], F32)
            bkv_sb = consts.tile([128, 2], F32)
            padb_sb = consts.tile([128, NT], F32)
            dmask_sb = consts.tile([128, 4 * SC], BF16)

            # Persistent activations (live across stages)
            QT = persist.tile([128, HPC, S], BF16)   # Q^T per head: [d, h, q]
            KT = persist.tile([128, S], BF16)        # K^T: [d, kv]
            V = persist.tile([128, NT, 128], F16)    # V tiles: [kv_p, kv_tile, d]
            OT = persist.tile([128, HPC, S], BF16)   # (exp(S) V)^T normalized

            with (
                tc.tile_pool(name="w1", bufs=1) as w1p,
                tc.tile_pool(name="xt", bufs=20) as xtp,
                tc.tile_pool(name="vt", bufs=2) as vtp,
                tc.tile_pool(name="es", bufs=4) as esp,
                tc.tile_pool(name="acc", bufs=2) as accp,
                tc.tile_pool(name="rs", bufs=2) as rsp,
                tc.tile_pool(name="bb", bufs=2) as bbp,
                tc.tile_pool(name="outsb", bufs=4) as outp,
                tc.tile_pool(name="psS", bufs=1, space="PSUM") as psS,
                tc.tile_pool(name="psO", bufs=1, space="PSUM") as psO,
            ):
                wk_sb = w1p.tile([128, NHT, D], BF16)
                nc.sync.dma_start(wk_sb[:], wk[:])
                wv_sb = w1p.tile([128, NHT, D], BF16)
                wq_sb = w1p.tile([128, HPC, NHT, 128], BF16)
                wo_sb = w1p.tile([128, HPC, HID], BF16)

                pending = None
                for sc in range(NSC):
                    # ---------------- stage 1: projections for chunk sc ----
                    xtc = xtp.tile([128, NHT, SC], BF16, tag="xt", bufs=2)
                    if sc == 0:
                        nc.sync.dma_start(xtc[:, 0:4], xT[sc, :, 0:4])
                        nc.sync.dma_start(xtc[:, 4:NHT], xT[sc, :, 4:NHT])
                    else:
                        nc.sync.dma_start(xtc[:], xT[sc])
                    xts = [xtc[:, ht, :] for ht in range(NHT)]
                    if sc == 0:
                        nc.sync.dma_start(wv_sb[:], wv[:])
                        nc.sync.dma_start(bkv_sb[:], bkv[:])
                        nc.sync.dma_start(wq_sb[:, 0], wq[0])
                        nc.sync.dma_start(wq_sb[:, 1], wq[1])
                        nc.sync.dma_start(bq_sb[:], bq[:])
                        nc.sync.dma_start(wq_sb[:, 2], wq[2])
                        nc.sync.dma_start(wq_sb[:, 3], wq[3])
                        nc.sync.dma_start(padb_sb[:], padb[:])
                        nc.sync.dma_start(dmask_sb[:], dmask[:])
                    elif sc == 1:
                        nc.sync.dma_start(wo_sb[:], wo[:])

                    if pending is not None:
                        p_qc, p_psos, p_accA, p_accB = pending
                        psrA = psS.tile(
                            [128, 1024], F32, tag="sA", name="psrA"
                        )
                        for hh in range(2):
                            nc.tensor.matmul(
                                psrA[0:1, ts(hh, SC)],
                                ones_col[:], p_accA[:, ts(hh, SC)],
                                start=True, stop=True,
                            )
                        rsA = rsp.tile([1, 1024], F32, tag="rA", name="rsA")
                        for hh in range(2):
                            nc.vector.reciprocal_approx_fast(
                                rsA[:, ts(hh, SC)], psrA[0:1, ts(hh, SC)]
                            )

                    pskv = psS.tile([128, 1024], F32, tag="sB", name="pskv")
                    for ht in range(NHT):
                        nc.tensor.matmul(
                            pskv[:, 0:SC], wk_sb[:, ht, :], xts[ht],
                            start=(ht == 0), stop=(ht == NHT - 1),
                        )
                    nc.scalar.activation(
                        KT[:, ts(sc, SC)], pskv[:, 0:SC], Ident,
                        bias=bkv_sb[:, 0:1],
                    )
                    for ht in range(NHT):
                        nc.tensor.matmul(
                            pskv[:, SC:1024], wv_sb[:, ht, :], xts[ht],
                            start=(ht == 0), stop=(ht == NHT - 1),
                        )
                    vt_s = vtp.tile([128, SC], F16, tag="vt")
                    nc.scalar.activation(
                        vt_s[:], pskv[:, SC:1024], Ident, bias=bkv_sb[:, 1:2]
                    )

                    if pending is not None:
                        psrB = psS.tile(
                            [128, 1024], F32, tag="sA", name="psrB"
                        )
                        for hh in range(2):
                            nc.tensor.matmul(
                                psrB[0:1, ts(hh, SC)],
                                ones_col[:], p_accB[:, ts(hh, SC)],
                                start=True, stop=True,
                            )
                        rsB = rsp.tile([1, 1024], F32, tag="rB", name="rsB")
                        for hh in range(2):
                            nc.vector.reciprocal_approx_fast(
                                rsB[:, ts(hh, SC)], psrB[0:1, ts(hh, SC)]
                            )
                        bbA = bbp.tile([128, 1024], F32, tag="bA", name="bbA")
                        bbB = bbp.tile([128, 1024], F32, tag="bB", name="bbB")
                        nc.gpsimd.partition_broadcast(bbA[:], rsA[:])
                        nc.gpsimd.partition_broadcast(bbB[:], rsB[:])
                        for h in range(HPC):
                            bb = bbA if h < 2 else bbB
                            nc.vector.tensor_mul(
                                OT[:, h, ts(p_qc, SC)], p_psos[h][:],
                                bb[:, ts(h % 2, SC)],
                            )
                        pending = None

                    psq01 = psS.tile([128, 1024], F32, tag="sB", name="psq01")
                    for dt in range(2):
                        for ht in range(NHT):
                            nc.tensor.matmul(
                                psq01[:, ts(dt, SC)],
                                wq_sb[:, dt, ht, :], xts[ht],
                                start=(ht == 0), stop=(ht == NHT - 1),
                            )
                        nc.scalar.activation(
                            QT[:, dt, ts(sc, SC)], psq01[:, ts(dt, SC)], Ident,
                            bias=bq_sb[:, dt : dt + 1],
                        )

                    psq23 = psS.tile([128, 1024], F32, tag="sA", name="psq23")
                    for dt in range(2):
                        for ht in range(NHT):
                            nc.tensor.matmul(
                                psq23[:, ts(dt, SC)],
                                wq_sb[:, 2 + dt, ht, :], xts[ht],
                                start=(ht == 0), stop=(ht == NHT - 1),
                            )
                        nc.scalar.activation(
                            QT[:, 2 + dt, ts(sc, SC)], psq23[:, ts(dt, SC)],
                            Ident, bias=bq_sb[:, 2 + dt : 3 + dt],
                        )

                    # V chunk transpose into [kv_p, kv_tile, d] tiles
                    pstr = psS.tile([128, SC], F16, tag="sB", name="pstr")
                    for j in range(SC // 128):
                        nc.tensor.transpose(
                            pstr[:, ts(j, 128)], vt_s[:, ts(j, 128)],
                            ident16[:],
                        )
                    nc.vector.tensor_copy(V[:, sc * 4 : sc * 4 + 4, :], pstr[:])

                    # ---------------- stage 2: attention for qc = sc -------
                    qc = sc
                    nkt = 4 * qc + 4
                    psos = [
                        psO.tile([128, SC], F32, tag=f"o{h}", name=f"pso_{h}")
                        for h in range(HPC)
                    ]
                    acc2A = accp.tile([128, 1024], F16, tag="aA", name="acc2A")
                    acc2B = accp.tile([128, 1024], F16, tag="aB", name="acc2B")
                    def sub2(ap1024, off):
                        # two-block view [128, 2, 512-off]: the valid causal
                        # q-suffix of both head-halves of a [128,1024] tile
                        return ap1024.rearrange("p (h f) -> p h f", h=2)[
                            :, :, off:
                        ]

                    prev = None
                    for kt in range(nkt):
                        j = kt - 4 * qc
                        diag = j >= 0
                        off = 128 * j if diag else 0
                        esA = esp.tile([128, 1024], F16, tag="esA", name="esA")
                        esB = esp.tile([128, 1024], F16, tag="esB", name="esB")
                        psa = psS.tile([128, 1024], F32, tag="sB", name="psa")
                        for hh in range(2):
                            nc.tensor.matmul(
                                psa[:, hh * SC + off : (hh + 1) * SC],
                                KT[:, ts(kt, 128)],
                                QT[:, hh, qc * SC + off : (qc + 1) * SC],
                                start=True, stop=not diag,
                            )
                        if diag:
                            for hh in range(2):
                                nc.tensor.matmul(
                                    psa[:, hh * SC + off : (hh + 1) * SC],
                                    ident[:],
                                    dmask_sb[:, j * SC + off : (j + 1) * SC],
                                    start=False, stop=True,
                                )
                        nc.scalar.activation(
                            sub2(esA, off), sub2(psa, off), Exp,
                            bias=padb_sb[:, kt : kt + 1], scale=SCALE,
                        )
                        psb = psS.tile([128, 1024], F32, tag="sA", name="psb")
                        for hh in range(2):
                            nc.tensor.matmul(
                                psb[:, hh * SC + off : (hh + 1) * SC],
                                KT[:, ts(kt, 128)],
                                QT[:, 2 + hh, qc * SC + off : (qc + 1) * SC],
                                start=True, stop=not diag,
                            )
                        if diag:
                            for hh in range(2):
                                nc.tensor.matmul(
                                    psb[:, hh * SC + off : (hh + 1) * SC],
                                    ident[:],
                                    dmask_sb[:, j * SC + off : (j + 1) * SC],
                                    start=False, stop=True,
                                )
                        nc.scalar.activation(
                            sub2(esB, off), sub2(psb, off), Exp,
                            bias=padb_sb[:, kt : kt + 1], scale=SCALE,
                        )
                        # AV matmuls for the previous kt (software
                        # pipeline), restricted to the valid causal
                        # q-suffix; every diagonal AV closes its region's
                        # accumulation group (stop=True) so the readiness
                        # semaphore for psos is placed correctly
                        if prev is not None:
                            pkt, peA, peB, poff = prev
                            for h in range(HPC):
                                pe = peA if h < 2 else peB
                                nc.tensor.matmul(
                                    psos[h][:, poff:SC], V[:, pkt, :],
                                    pe[:, (h % 2) * SC + poff
                                       : (h % 2 + 1) * SC],
                                    start=(pkt == 0), stop=(poff > 0),
                                    skip_group_check=(poff > 0),
                                )
                        # denominator accumulation (fp16, 2x DVE rate)
                        if kt == 0:
                            nc.vector.tensor_copy(acc2A[:], esA[:])
                            nc.vector.tensor_copy(acc2B[:], esB[:])
                        else:
                            nc.vector.tensor_add(
                                sub2(acc2A, off), sub2(acc2A, off),
                                sub2(esA, off),
                            )
                            nc.vector.tensor_add(
                                sub2(acc2B, off), sub2(acc2B, off),
                                sub2(esB, off),
                            )
                        prev = (kt, esA, esB, off)
                    pkt, peA, peB, poff = prev
                    for h in range(HPC):
                        pe = peA if h < 2 else peB
                        nc.tensor.matmul(
                            psos[h][:, poff:SC], V[:, pkt, :],
                            pe[:, (h % 2) * SC + poff : (h % 2 + 1) * SC],
                            start=(pkt == 0), stop=True,
                            skip_group_check=True,
                        )

                    pending = (qc, psos, acc2A, acc2B)

                # deferred normalization for the last chunk (qc = 3)
                p_qc, p_psos, p_accA, p_accB = pending
                psrA = psS.tile([128, 1024], F32, tag="sA", name="psrA")
                psrB = psS.tile([128, 1024], F32, tag="sB", name="psrB")
                for hh in range(2):
                    nc.tensor.matmul(
                        psrA[0:1, ts(hh, SC)],
                        ones_col[:], p_accA[:, ts(hh, SC)],
                        start=True, stop=True,
                    )
                    nc.tensor.matmul(
                        psrB[0:1, ts(hh, SC)],
                        ones_col[:], p_accB[:, ts(hh, SC)],
                        start=True, stop=True,
                    )
                rsA = rsp.tile([1, 1024], F32, tag="rA", name="rsA")
                rsB = rsp.tile([1, 1024], F32, tag="rB", name="rsB")
                for hh in range(2):
                    nc.vector.reciprocal_approx_fast(
                        rsA[:, ts(hh, SC)], psrA[0:1, ts(hh, SC)]
                    )
                    nc.vector.reciprocal_approx_fast(
                        rsB[:, ts(hh, SC)], psrB[0:1, ts(hh, SC)]
                    )
                bbA = bbp.tile([128, 1024], F32, tag="bA", name="bbA")
                bbB = bbp.tile([128, 1024], F32, tag="bB", name="bbB")
                nc.gpsimd.partition_broadcast(bbA[:], rsA[:])
                nc.gpsimd.partition_broadcast(bbB[:], rsB[:])
                for h in range(HPC):
                    bb = bbA if h < 2 else bbB
                    nc.vector.tensor_mul(
                        OT[:, h, ts(p_qc, SC)], p_psos[h][:],
                        bb[:, ts(h % 2, SC)],
                    )

                # ---------------- stage 3: output projection ----------------
                # 32 half-tiles (st, colhalf) alternating score tags so the
                # PSUM->SBUF copy of one overlaps the matmuls of the next.
                for sth in range(2 * NT):
                    st, half = sth // 2, sth % 2
                    ps3 = psS.tile(
                        [128, 1024], F32, tag=("sA" if sth % 2 == 0 else "sB"),
                        name="ps3",
                    )
                    for dt in range(HPC):
                        for hc in range(2):
                            nc.tensor.matmul(
                                ps3[:, ts(hc, SC)],
                                OT[:, dt, ts(st, 128)],
                                wo_sb[:, dt, ts(2 * half + hc, SC)],
                                start=(dt == 0), stop=(dt == HPC - 1),
                            )
                    ot = outp.tile([128, 1024], BF16, tag=f"out{half}")
                    if half == 0:
                        nc.scalar.copy(ot[:], ps3[:])
                    else:
                        nc.vector.tensor_copy(ot[:], ps3[:])
                    nc.sync.dma_start(out[ts(st, 128), ts(half, 1024)], ot[:])
    nc.compile()
    return nc


def _get_program():
    global _PROGRAM
    if _PROGRAM is None:
        _PROGRAM = _build_program()
    return _PROGRAM


def kernel(**inputs):
    global LAST_RESULT
    hs = np.ascontiguousarray(inputs["hidden_states"], dtype=np.float32)
    pad = np.ascontiguousarray(inputs["padding_mask"], dtype=np.float32)
    Wq = np.asarray(inputs["Wq"], dtype=np.float32)
    Wk = np.asarray(inputs["Wk"], dtype=np.float32)
    Wv = np.asarray(inputs["Wv"], dtype=np.float32)
    Wo = np.asarray(inputs["Wo"], dtype=np.float32)
    bq_v = np.asarray(inputs["bq"], dtype=np.float32)
    bk_v = np.asarray(inputs["bk"], dtype=np.float32)
    bv_v = np.asarray(inputs["bv"], dtype=np.float32)
    bo_v = np.asarray(inputs["bo"], dtype=np.float32)

    xTs = [
        np.ascontiguousarray(
            hs[b].T.reshape(NHT, 128, NSC, SC).transpose(2, 1, 0, 3)
        ).astype(NP_BF16)
        for b in range(B)
    ]
    WqT = Wq.T  # [HID, HID]
    WkT = np.ascontiguousarray(Wk.T).astype(NP_BF16)  # [HID, D]
    WvT = np.ascontiguousarray(Wv.T).astype(NP_BF16)
    WoT = Wo.T  # [HID, HID]

    # 4 diagonal-block masks in S^T layout: mask_j[p, f] = 0 if p+128*j <= f
    p_i = np.arange(128)[:, None]
    f_i = np.arange(SC)[None, :]
    dmask = np.empty((128, 4 * SC), np.float32)
    for j in range(4):
        dmask[:, j * SC : (j + 1) * SC] = np.where(
            p_i + 128 * j <= f_i, 0.0, NEG
        ).astype(np.float32)
    dmask = dmask.astype(NP_BF16)

    padbs = [
        np.ascontiguousarray((NEG * pad[b]).reshape(NT, 128).T) for b in range(B)
    ]
    bqs = [
        np.ascontiguousarray(
            bq_v[hg * DPH : (hg + 1) * DPH].reshape(HPC, 128).T
        )
        for hg in range(HPC)
    ]
    bkv = np.ascontiguousarray(np.stack([bk_v, bv_v], axis=1))  # [128, 2]

    wk_l = np.ascontiguousarray(
        WkT.reshape(NHT, 128, D).transpose(1, 0, 2)
    ).astype(NP_BF16)
    wv_l = np.ascontiguousarray(
        WvT.reshape(NHT, 128, D).transpose(1, 0, 2)
    ).astype(NP_BF16)
    wq_l = [
        np.ascontiguousarray(
            WqT[:, hg * DPH : (hg + 1) * DPH]
            .reshape(NHT, 128, HPC, 128)
            .transpose(2, 1, 0, 3)
        ).astype(NP_BF16)
        for hg in range(HPC)
    ]
    wo_l = [
        np.ascontiguousarray(
            WoT[hg * DPH : (hg + 1) * DPH, :]
            .reshape(HPC, 128, HID)
            .transpose(1, 0, 2)
        ).astype(NP_BF16)
        for hg in range(HPC)
    ]

    nc = _get_program()
    in_maps = []
    for c in range(NCORES):
        b, hg = c // 4, c % 4
        in_maps.append(
            {
                "xT": xTs[b],
                "wq": wq_l[hg],
                "wk": wk_l,
                "wv": wv_l,
                "wo": wo_l[hg],
                "bq": bqs[hg],
                "bkv": bkv,
                "padb": padbs[b],
                "dmask": dmask,
            }
        )

    LAST_RESULT = run_bass_kernel_spmd(nc, in_maps, list(range(NCORES)))
    res = LAST_RESULT.results

    outp = np.zeros((B, S, HID), np.float32)
    for c in range(NCORES):
        outp[c // 4] += res[c]["out"].astype(np.float32)
    outp += bo_v[None, None, :]
    return outp


if __name__ == "__main__":
    rng = np.random.default_rng(0)
    demo = {
        "hidden_states": rng.standard_normal((B, S, HID), dtype=np.float32),
        "causal_mask": np.triu(np.ones((1, 1, S, S), np.float32), k=1),
        "padding_mask": np.zeros((B, S), np.float32),
        "Wq": (rng.standard_normal((HID, HID), dtype=np.float32) * 0.02),
        "bq": np.zeros((HID,), np.float32),
        "Wk": (rng.standard_normal((D, HID), dtype=np.float32) * 0.02),
        "bk": np.zeros((D,), np.float32),
        "Wv": (rng.standard_normal((D, HID), dtype=np.float32) * 0.02),
        "bv": np.zeros((D,), np.float32),
        "Wo": (rng.standard_normal((HID, HID), dtype=np.float32) * 0.02),
        "bo": np.zeros((HID,), np.float32),
    }
    o = kernel(**demo)
    print("kernel output", o.shape, o.dtype, float(np.abs(o).mean()))


# revision 31
# speedup vs baseline: 1.0110x; 1.0110x over previous
"""MQA (GQA with 1 KV group) attention kernel for 8 Trainium2 NeuronCores.

Sharding: core c -> batch b = c//4, head-group hg = c%4 (4 of 16 query heads).
Each core computes Q/K/V projections from x[b]^T, causal attention for its 4
heads in transposed layout (S^T[kv, q] tiles), and a partial output
projection out_partial = A_h @ Wo[:, cols_h]^T.  Host sums the 4 partials per
batch and adds bo.

Structure (one pass over 4 s-chunks, fully interleaved emission so every
engine's queue overlaps the PE-bound projection work):
  - proj chunk sc (K,V -> one 2-bank PSUM tile; Q pairs likewise), then
    attention qc=sc, with the qc-1 normalization deferred into chunk sc's
    emission so no PSUM tag is blocked at chunk boundaries;
  - scores live in [128,1024] two-bank PSUM tiles (one exp per head-pair,
    same KT/V stationary reused across the pair);
  - the causal mask is accumulated onto scores by the PE (identity matmul)
    and score/exp/accumulate work is restricted to the valid causal
    q-suffix on diagonal tiles (GpSimd zeroes the dead es prefix);
  - exp outputs / softmax denominators are fp16 (2x DVE rate, full-rate
    rowsum matmul); 1/rowsum via reciprocal_approx_fast; the per-q scale is
    partition-broadcast on GpSimd.  All PSUM rowsums stay at partition 0:
    DVE lanes are partition-locked on HW, and gpsimd.partition_broadcast
    reads absolute partition 0 (offset-base APs silently break on HW even
    though CoreSim accepts them);
  - output projection runs as 32 half-tiles alternating the two score tags,
    bf16 partials summed on host;
  - weights are host-pre-transposed into per-partition-contiguous layouts
    (strided rearrange DMAs cost ~4x in descriptor overhead), and the wq
    chunks queue behind the first xT chunk so nothing stalls startup.
"""

import sys

sys.path.insert(0, "/opt/trn_rl_repo")

import ml_dtypes
import numpy as np

import concourse.bass as bass
import concourse.tile as tile
from concourse import bacc
from concourse import mybir
from concourse.bass import ts
from concourse.bass_utils import run_bass_kernel_spmd
from concourse.masks import make_identity

B, S, HID = 2, 2048, 2048
H, D = 16, 128
HPC = 4              # heads per core
DPH = HPC * D        # 512: head dims per core
NCORES = 8
SC = 512             # s-chunk (free dim for most matmuls)
NSC = S // SC        # 4
NT = S // 128        # 16 128-tiles along s / hid
NHT = HID // 128     # 16 hid tiles
SCALE = 1.0 / float(np.sqrt(D))
NEG = -1.0e9

F32 = mybir.dt.float32
F16 = mybir.dt.float16
BF16 = mybir.dt.bfloat16
NP_BF16 = ml_dtypes.bfloat16

_PROGRAM = None
LAST_RESULT = None


def _build_program():
    nc = bacc.Bacc()
    xT = nc.declare_dram_parameter("xT", [NSC, 128, NHT, SC], BF16, isOutput=False)
    wq = nc.declare_dram_parameter("wq", [HPC, 128, NHT, 128], BF16, isOutput=False)
    wk = nc.declare_dram_parameter("wk", [128, NHT, D], BF16, isOutput=False)
    wv = nc.declare_dram_parameter("wv", [128, NHT, D], BF16, isOutput=False)
    wo = nc.declare_dram_parameter("wo", [128, HPC, HID], BF16, isOutput=False)
    bq = nc.declare_dram_parameter("bq", [128, HPC], F32, isOutput=False)
    bkv = nc.declare_dram_parameter("bkv", [128, 2], F32, isOutput=False)
    padb = nc.declare_dram_parameter("padb", [128, NT], F32, isOutput=False)
    dmask = nc.declare_dram_parameter("dmask", [128, 4 * SC], BF16, isOutput=False)
    out = nc.declare_dram_parameter("out", [S, HID], BF16, isOutput=True)

    Exp = mybir.ActivationFunctionType.Exp
    Ident = mybir.ActivationFunctionType.Identity

    with tile.TileContext(nc) as tc:
        with (
            tc.tile_pool(name="consts", bufs=1) as consts,
            tc.tile_pool(name="persist", bufs=1) as persist,
        ):
            ident = consts.tile([128, 128], BF16)
            make_identity(nc, ident[:])
            ident16 = consts.tile([128, 128], F16)
            make_identity(nc, ident16[:])
            ones_col = consts.tile([128, 1], F16)
            nc.vector.memset(ones_col[:], 1.0)
            bq_sb = consts.tile([128, HPC], F32)
            bkv_sb = consts.tile([128, 2], F32)
            padb_sb = consts.tile([128, NT], F32)
            dmask_sb = consts.tile([128, 4 * SC], BF16)

            # Persistent activations (live across stages)
            QT = persist.tile([128, HPC, S], BF16)   # Q^T per head: [d, h, q]
            KT = persist.tile([128, S], BF16)        # K^T: [d, kv]
            V = persist.tile([128, NT, 128], F16)    # V tiles: [kv_p, kv_tile, d]
            OT = persist.tile([128, HPC, S], BF16)   # (exp(S) V)^T normalized

            with (
                tc.tile_pool(name="w1", bufs=1) as w1p,
                tc.tile_pool(name="xt", bufs=20) as xtp,
                tc.tile_pool(name="vt", bufs=2) as vtp,
                tc.tile_pool(name="es", bufs=4) as esp,
                tc.tile_pool(name="acc", bufs=2) as accp,
                tc.tile_pool(name="rs", bufs=2) as rsp,
                tc.tile_pool(name="bb", bufs=2) as bbp,
                tc.tile_pool(name="outsb", bufs=4) as outp,
                tc.tile_pool(name="psS", bufs=1, space="PSUM") as psS,
                tc.tile_pool(name="psO", bufs=1, space="PSUM") as psO,
            ):
                wk_sb = w1p.tile([128, NHT, D], BF16)
                nc.sync.dma_start(wk_sb[:], wk[:])
                wv_sb = w1p.tile([128, NHT, D], BF16)
                wq_sb = w1p.tile([128, HPC, NHT, 128], BF16)
                wo_sb = w1p.tile([128, HPC, HID], BF16)

                pending = None
                for sc in range(NSC):
                    # ---------------- stage 1: projections for chunk sc ----
                    xtc = xtp.tile([128, NHT, SC], BF16, tag="xt", bufs=2)
                    if sc == 0:
                        nc.sync.dma_start(xtc[:, 0:8], xT[sc, :, 0:8])
                        nc.sync.dma_start(xtc[:, 8:NHT], xT[sc, :, 8:NHT])
                    else:
                        nc.sync.dma_start(xtc[:], xT[sc])
                    xts = [xtc[:, ht, :] for ht in range(NHT)]
                    if sc == 0:
                        nc.sync.dma_start(wv_sb[:], wv[:])
                        nc.sync.dma_start(bkv_sb[:], bkv[:])
                        nc.sync.dma_start(wq_sb[:, 0], wq[0])
                        nc.sync.dma_start(wq_sb[:, 1], wq[1])
                        nc.sync.dma_start(bq_sb[:], bq[:])
                        nc.sync.dma_start(wq_sb[:, 2], wq[2])
                        nc.sync.dma_start(wq_sb[:, 3], wq[3])
                        nc.sync.dma_start(padb_sb[:], padb[:])
                        nc.sync.dma_start(dmask_sb[:], dmask[:])
                    elif sc == 1:
                        nc.sync.dma_start(wo_sb[:], wo[:])

                    if pending is not None:
                        p_qc, p_psos, p_accA, p_accB = pending
                        psrA = psS.tile(
                            [128, 1024], F32, tag="sA", name="psrA"
                        )
                        for hh in range(2):
                            nc.tensor.matmul(
                                psrA[0:1, ts(hh, SC)],
                                ones_col[:], p_accA[:, ts(hh, SC)],
                                start=True, stop=True,
                            )
                        rsA = rsp.tile([1, 1024], F32, tag="rA", name="rsA")
                        for hh in range(2):
                            nc.vector.reciprocal_approx_fast(
                                rsA[:, ts(hh, SC)], psrA[0:1, ts(hh, SC)]
                            )

                    pskv = psS.tile([128, 1024], F32, tag="sB", name="pskv")
                    for ht in range(NHT):
                        nc.tensor.matmul(
                            pskv[:, 0:SC], wk_sb[:, ht, :], xts[ht],
                            start=(ht == 0), stop=(ht == NHT - 1),
                        )
                    nc.scalar.activation(
                        KT[:, ts(sc, SC)], pskv[:, 0:SC], Ident,
                        bias=bkv_sb[:, 0:1],
                    )
                    for ht in range(NHT):
                        nc.tensor.matmul(
                            pskv[:, SC:1024], wv_sb[:, ht, :], xts[ht],
                            start=(ht == 0), stop=(ht == NHT - 1),
                        )
                    vt_s = vtp.tile([128, SC], F16, tag="vt")
                    nc.scalar.activation(
                        vt_s[:], pskv[:, SC:1024], Ident, bias=bkv_sb[:, 1:2]
                    )

                    if pending is not None:
                        psrB = psS.tile(
                            [128, 1024], F32, tag="sA", name="psrB"
                        )
                        for hh in range(2):
                            nc.tensor.matmul(
                                psrB[0:1, ts(hh, SC)],
                                ones_col[:], p_accB[:, ts(hh, SC)],
                                start=True, stop=True,
                            )
                        rsB = rsp.tile([1, 1024], F32, tag="rB", name="rsB")
                        for hh in range(2):
                            nc.vector.reciprocal_approx_fast(
                                rsB[:, ts(hh, SC)], psrB[0:1, ts(hh, SC)]
                            )
                        bbA = bbp.tile([128, 1024], F32, tag="bA", name="bbA")
                        bbB = bbp.tile([128, 1024], F32, tag="bB", name="bbB")
                        nc.gpsimd.partition_broadcast(bbA[:], rsA[:])
                        nc.gpsimd.partition_broadcast(bbB[:], rsB[:])
                        for h in range(HPC):
                            bb = bbA if h < 2 else bbB
                            nc.vector.tensor_mul(
                                OT[:, h, ts(p_qc, SC)], p_psos[h][:],
                                bb[:, ts(h % 2, SC)],
                            )
                        pending = None

                    psq01 = psS.tile([128, 1024], F32, tag="sB", name="psq01")
                    for dt in range(2):
                        for ht in range(NHT):
                            nc.tensor.matmul(
                                psq01[:, ts(dt, SC)],
                                wq_sb[:, dt, ht, :], xts[ht],
                                start=(ht == 0), stop=(ht == NHT - 1),
                            )
                        nc.scalar.activation(
                            QT[:, dt, ts(sc, SC)], psq01[:, ts(dt, SC)], Ident,
                            bias=bq_sb[:, dt : dt + 1],
                        )

                    psq23 = psS.tile([128, 1024], F32, tag="sA", name="psq23")
                    for dt in range(2):
                        for ht in range(NHT):
                            nc.tensor.matmul(
                                psq23[:, ts(dt, SC)],
                                wq_sb[:, 2 + dt, ht, :], xts[ht],
                                start=(ht == 0), stop=(ht == NHT - 1),
                            )
                        nc.scalar.activation(
                            QT[:, 2 + dt, ts(sc, SC)], psq23[:, ts(dt, SC)],
                            Ident, bias=bq_sb[:, 2 + dt : 3 + dt],
                        )

                    # V chunk transpose into [kv_p, kv_tile, d] tiles
                    pstr = psS.tile([128, SC], F16, tag="sB", name="pstr")
                    for j in range(SC // 128):
                        nc.tensor.transpose(
                            pstr[:, ts(j, 128)], vt_s[:, ts(j, 128)],
                            ident16[:],
                        )
                    nc.vector.tensor_copy(V[:, sc * 4 : sc * 4 + 4, :], pstr[:])

                    # ---------------- stage 2: attention for qc = sc -------
                    qc = sc
                    nkt = 4 * qc + 4
                    psos = [
                        psO.tile([128, SC], F32, tag=f"o{h}", name=f"pso_{h}")
                        for h in range(HPC)
                    ]
                    acc2A = accp.tile([128, 1024], F16, tag="aA", name="acc2A")
                    acc2B = accp.tile([128, 1024], F16, tag="aB", name="acc2B")
                    def sub2(ap1024, off):
                        # two-block view [128, 2, 512-off]: the valid causal
                        # q-suffix of both head-halves of a [128,1024] tile
                        return ap1024.rearrange("p (h f) -> p h f", h=2)[
                            :, :, off:
                        ]

                    prev = None
                    for kt in range(nkt):
                        j = kt - 4 * qc
                        diag = j >= 0
                        off = 128 * j if diag else 0
                        esA = esp.tile([128, 1024], F16, tag="esA", name="esA")
                        esB = esp.tile([128, 1024], F16, tag="esB", name="esB")
                        psa = psS.tile([128, 1024], F32, tag="sB", name="psa")
                        for hh in range(2):
                            nc.tensor.matmul(
                                psa[:, hh * SC + off : (hh + 1) * SC],
                                KT[:, ts(kt, 128)],
                                QT[:, hh, qc * SC + off : (qc + 1) * SC],
                                start=True, stop=not diag,
                            )
                        if diag:
                            for hh in range(2):
                                nc.tensor.matmul(
                                    psa[:, hh * SC + off : (hh + 1) * SC],
                                    ident[:],
                                    dmask_sb[:, j * SC + off : (j + 1) * SC],
                                    start=False, stop=True,
                                )
                        nc.scalar.activation(
                            sub2(esA, off), sub2(psa, off), Exp,
                            bias=padb_sb[:, kt : kt + 1], scale=SCALE,
                        )
                        psb = psS.tile([128, 1024], F32, tag="sA", name="psb")
                        for hh in range(2):
                            nc.tensor.matmul(
                                psb[:, hh * SC + off : (hh + 1) * SC],
                                KT[:, ts(kt, 128)],
                                QT[:, 2 + hh, qc * SC + off : (qc + 1) * SC],
                                start=True, stop=not diag,
                            )
                        if diag:
                            for hh in range(2):
                                nc.tensor.matmul(
                                    psb[:, hh * SC + off : (hh + 1) * SC],
                                    ident[:],
                                    dmask_sb[:, j * SC + off : (j + 1) * SC],
                                    start=False, stop=True,
                                )
                        nc.scalar.activation(
                            sub2(esB, off), sub2(psb, off), Exp,
                            bias=padb_sb[:, kt : kt + 1], scale=SCALE,
                        )
                        # AV matmuls for the previous kt (software
                        # pipeline), restricted to the valid causal
                        # q-suffix; every diagonal AV closes its region's
                        # accumulation group (stop=True) so the readiness
                        # semaphore for psos is placed correctly
                        if prev is not None:
                            pkt, peA, peB, poff = prev
                            for h in range(HPC):
                                pe = peA if h < 2 else peB
                                nc.tensor.matmul(
                                    psos[h][:, poff:SC], V[:, pkt, :],
                                    pe[:, (h % 2) * SC + poff
                                       : (h % 2 + 1) * SC],
                                    start=(pkt == 0), stop=(poff > 0),
                                    skip_group_check=(poff > 0),
                                )
                        # denominator accumulation (fp16, 2x DVE rate)
                        if kt == 0:
                            nc.vector.tensor_copy(acc2A[:], esA[:])
                            nc.vector.tensor_copy(acc2B[:], esB[:])
                        else:
                            nc.vector.tensor_add(
                                sub2(acc2A, off), sub2(acc2A, off),
                                sub2(esA, off),
                            )
                            nc.vector.tensor_add(
                                sub2(acc2B, off), sub2(acc2B, off),
                                sub2(esB, off),
                            )
                        prev = (kt, esA, esB, off)
                    pkt, peA, peB, poff = prev
                    for h in range(HPC):
                        pe = peA if h < 2 else peB
                        nc.tensor.matmul(
                            psos[h][:, poff:SC], V[:, pkt, :],
                            pe[:, (h % 2) * SC + poff : (h % 2 + 1) * SC],
                            start=(pkt == 0), stop=True,
                            skip_group_check=True,
                        )

                    pending = (qc, psos, acc2A, acc2B)

                # deferred normalization for the last chunk (qc = 3)
                p_qc, p_psos, p_accA, p_accB = pending
                psrA = psS.tile([128, 1024], F32, tag="sA", name="psrA")
                psrB = psS.tile([128, 1024], F32, tag="sB", name="psrB")
                for hh in range(2):
                    nc.tensor.matmul(
                        psrA[0:1, ts(hh, SC)],
                        ones_col[:], p_accA[:, ts(hh, SC)],
                        start=True, stop=True,
                    )
                    nc.tensor.matmul(
                        psrB[0:1, ts(hh, SC)],
                        ones_col[:], p_accB[:, ts(hh, SC)],
                        start=True, stop=True,
                    )
                rsA = rsp.tile([1, 1024], F32, tag="rA", name="rsA")
                rsB = rsp.tile([1, 1024], F32, tag="rB", name="rsB")
                for hh in range(2):
                    nc.vector.reciprocal_approx_fast(
                        rsA[:, ts(hh, SC)], psrA[0:1, ts(hh, SC)]
                    )
                    nc.vector.reciprocal_approx_fast(
                        rsB[:, ts(hh, SC)], psrB[0:1, ts(hh, SC)]
                    )
                bbA = bbp.tile([128, 1024], F32, tag="bA", name="bbA")
                bbB = bbp.tile([128, 1024], F32, tag="bB", name="bbB")
                nc.gpsimd.partition_broadcast(bbA[:], rsA[:])
                nc.gpsimd.partition_broadcast(bbB[:], rsB[:])
                for h in range(HPC):
                    bb = bbA if h < 2 else bbB
                    nc.vector.tensor_mul(
                        OT[:, h, ts(p_qc, SC)], p_psos[h][:],
                        bb[:, ts(h % 2, SC)],
                    )

                # ---------------- stage 3: output projection ----------------
                # 32 half-tiles (st, colhalf) alternating score tags so the
                # PSUM->SBUF copy of one overlaps the matmuls of the next.
                for sth in range(2 * NT):
                    st, half = sth // 2, sth % 2
                    ps3 = psS.tile(
                        [128, 1024], F32, tag=("sA" if sth % 2 == 0 else "sB"),
                        name="ps3",
                    )
                    for dt in range(HPC):
                        for hc in range(2):
                            nc.tensor.matmul(
                                ps3[:, ts(hc, SC)],
                                OT[:, dt, ts(st, 128)],
                                wo_sb[:, dt, ts(2 * half + hc, SC)],
                                start=(dt == 0), stop=(dt == HPC - 1),
                            )
                    ot = outp.tile([128, 1024], BF16, tag=f"out{half}")
                    if half == 0:
                        nc.scalar.copy(ot[:], ps3[:])
                    else:
                        nc.vector.tensor_copy(ot[:], ps3[:])
                    nc.sync.dma_start(out[ts(st, 128), ts(half, 1024)], ot[:])
    nc.compile()
    return nc


def _get_program():
    global _PROGRAM
    if _PROGRAM is None:
        _PROGRAM = _build_program()
    return _PROGRAM


def kernel(**inputs):
    global LAST_RESULT
    hs = np.ascontiguousarray(inputs["hidden_states"], dtype=np.float32)
    pad = np.ascontiguousarray(inputs["padding_mask"], dtype=np.float32)
    Wq = np.asarray(inputs["Wq"], dtype=np.float32)
    Wk = np.asarray(inputs["Wk"], dtype=np.float32)
    Wv = np.asarray(inputs["Wv"], dtype=np.float32)
    Wo = np.asarray(inputs["Wo"], dtype=np.float32)
    bq_v = np.asarray(inputs["bq"], dtype=np.float32)
    bk_v = np.asarray(inputs["bk"], dtype=np.float32)
    bv_v = np.asarray(inputs["bv"], dtype=np.float32)
    bo_v = np.asarray(inputs["bo"], dtype=np.float32)

    xTs = [
        np.ascontiguousarray(
            hs[b].T.reshape(NHT, 128, NSC, SC).transpose(2, 1, 0, 3)
        ).astype(NP_BF16)
        for b in range(B)
    ]
    WqT = Wq.T  # [HID, HID]
    WkT = np.ascontiguousarray(Wk.T).astype(NP_BF16)  # [HID, D]
    WvT = np.ascontiguousarray(Wv.T).astype(NP_BF16)
    WoT = Wo.T  # [HID, HID]

    # 4 diagonal-block masks in S^T layout: mask_j[p, f] = 0 if p+128*j <= f
    p_i = np.arange(128)[:, None]
    f_i = np.arange(SC)[None, :]
    dmask = np.empty((128, 4 * SC), np.float32)
    for j in range(4):
        dmask[:, j * SC : (j + 1) * SC] = np.where(
            p_i + 128 * j <= f_i, 0.0, NEG
        ).astype(np.float32)
    dmask = dmask.astype(NP_BF16)

    padbs = [
        np.ascontiguousarray((NEG * pad[b]).reshape(NT, 128).T) for b in range(B)
    ]
    bqs = [
        np.ascontiguousarray(
            bq_v[hg * DPH : (hg + 1) * DPH].reshape(HPC, 128).T
        )
        for hg in range(HPC)
    ]
    bkv = np.ascontiguousarray(np.stack([bk_v, bv_v], axis=1))  # [128, 2]

    wk_l = np.ascontiguousarray(
        WkT.reshape(NHT, 128, D).transpose(1, 0, 2)
    ).astype(NP_BF16)
    wv_l = np.ascontiguousarray(
        WvT.reshape(NHT, 128, D).transpose(1, 0, 2)
    ).astype(NP_BF16)
    wq_l = [
        np.ascontiguousarray(
            WqT[:, hg * DPH : (hg + 1) * DPH]
            .reshape(NHT, 128, HPC, 128)
            .transpose(2, 1, 0, 3)
        ).astype(NP_BF16)
        for hg in range(HPC)
    ]
    wo_l = [
        np.ascontiguousarray(
            WoT[hg * DPH : (hg + 1) * DPH, :]
            .reshape(HPC, 128, HID)
            .transpose(1, 0, 2)
        ).astype(NP_BF16)
        for hg in range(HPC)
    ]

    nc = _get_program()
    in_maps = []
    for c in range(NCORES):
        b, hg = c // 4, c % 4
        in_maps.append(
            {
                "xT": xTs[b],
                "wq": wq_l[hg],
                "wk": wk_l,
                "wv": wv_l,
                "wo": wo_l[hg],
                "bq": bqs[hg],
                "bkv": bkv,
                "padb": padbs[b],
                "dmask": dmask,
            }
        )

    LAST_RESULT = run_bass_kernel_spmd(nc, in_maps, list(range(NCORES)))
    res = LAST_RESULT.results

    outp = np.zeros((B, S, HID), np.float32)
    for c in range(NCORES):
        outp[c // 4] += res[c]["out"].astype(np.float32)
    outp += bo_v[None, None, :]
    return outp


if __name__ == "__main__":
    rng = np.random.default_rng(0)
    demo = {
        "hidden_states": rng.standard_normal((B, S, HID), dtype=np.float32),
        "causal_mask": np.triu(np.ones((1, 1, S, S), np.float32), k=1),
        "padding_mask": np.zeros((B, S), np.float32),
        "Wq": (rng.standard_normal((HID, HID), dtype=np.float32) * 0.02),
        "bq": np.zeros((HID,), np.float32),
        "Wk": (rng.standard_normal((D, HID), dtype=np.float32) * 0.02),
        "bk": np.zeros((D,), np.float32),
        "Wv": (rng.standard_normal((D, HID), dtype=np.float32) * 0.02),
        "bv": np.zeros((D,), np.float32),
        "Wo": (rng.standard_normal((HID, HID), dtype=np.float32) * 0.02),
        "bo": np.zeros((HID,), np.float32),
    }
    o = kernel(**demo)
    print("kernel output", o.shape, o.dtype, float(np.abs(o).mean()))
